# revision 1
# baseline (speedup 1.0000x reference)
"""Contrastive loss on 8 Trainium2 NeuronCores (Bass/Tile).

loss * n = sum_ij [ same_ij * (s<1)(1-s) + (1-same_ij) * (s>0.3) * s ],
s = <x_i, x_j>.

Decomposition used here (exact):
    loss * n = sum_ij b(s) + sum_ij same_ij * (relu(1-s) - b(s)),
    b(s) = (s > 0.3) * s.

Strategy:
  * Host: sort rows by label -> same-label pairs live in a narrow diagonal
    band (|i-j| < maxrun). Cast X^T to bf16.
  * Shard rows across 8 cores (1024 rows each). Each core receives a
    column-ROLLED copy of X^T so its own row-slab is always at columns
    0..1023 -> one SPMD program for all cores.
  * Device: S-slab [1024, 8192] via bf16 matmuls (PSUM fp32). Each
    [128,1024] S tile is copied PSUM->SBUF as bf16 (copies split between
    ScalarE and VectorE for engine balance), then one fused DVE op
    (scalar_tensor_tensor) computes b = (S>margin)*S with an accumulated
    per-row sum. Same-label corrections run only on the few band tiles
    straddling the diagonal, using an exact label-equality mask.
  * Host: fp64 sum of per-core accumulator vectors, divide by n.
"""

import numpy as np
import ml_dtypes

import concourse.bass as bass
import concourse.mybir as mybir
from concourse import bacc
import concourse.tile as tile
from concourse.bass_utils import run_bass_kernel_spmd

N_TOTAL = 8192
D = 256
N_CORES = 8
ROWS = N_TOTAL // N_CORES          # 1024 rows per core
M_TILES = ROWS // 128              # 8 partition tiles per core
DT_W = 1024                        # "double tile": 2 PSUM banks wide
N_DT = N_TOTAL // DT_W             # 8 double tiles across columns
MARGIN = 0.3
F32 = mybir.dt.float32
BF16 = mybir.dt.bfloat16

# number of (of 64) S double-tiles handled entirely on ScalarE via
# relu+sign accumulation (no SBUF copy, no DVE work). The rest get an
# ScalarE PSUM->SBUF copy + one fused DVE op. Tuned for engine balance.
RELU_TILES = 8


def _band_windows(pad):
    """Band windows in rolled column space, one entry per (mt, dt) slice:
    (mt, dt, lo, w, tcb_off). tcb region A = cols [0, 1024+pad),
    region B = cols [N-pad, N) stored at offset 1024+pad."""
    a_len = DT_W + pad
    wins = []
    for mt in range(M_TILES):
        c0 = mt * 128 - pad
        c1 = mt * 128 + 128 + pad
        ivs = []
        if c0 < 0:
            ivs.append((N_TOTAL + c0, N_TOTAL))
            c0 = 0
        ivs.append((c0, c1))
        for a, b in ivs:
            for dt in range(a // DT_W, (b - 1) // DT_W + 1):
                lo = max(a, dt * DT_W) - dt * DT_W
                hi = min(b, (dt + 1) * DT_W) - dt * DT_W
                col = dt * DT_W + lo
                if col < a_len:
                    tco = col
                else:
                    assert col >= N_TOTAL - pad
                    tco = a_len + (col - (N_TOTAL - pad))
                wins.append((mt, dt, lo, hi - lo, tco))
    return wins, a_len


def _main_body(nc, tc, psum, spool, bpool, wpool, xk, tcb, trows, accD,
               accE, bias_nm, rset, stt_col, colR, colS, colA, colB,
               wins_by_td, AL, ACT):
    for mt in range(M_TILES):
        lhs = [xk[k][:, mt * 128:(mt + 1) * 128] for k in range(2)]
        for g in range(N_DT // 2):
            dts = (2 * g, 2 * g + 1)
            T = [psum.tile([128, DT_W], F32, name="S") for _ in range(2)]
            for k in range(2):
                for j in range(2):
                    for h in range(2):
                        ntc = dts[j] * DT_W + h * 512
                        nc.tensor.matmul(
                            T[j][:, h * 512:(h + 1) * 512],
                            lhs[k],
                            xk[k][:, ntc:ntc + 512],
                            start=(k == 0),
                            stop=(k == 1),
                        )
            for j in range(2):
                td = (mt, dts[j])
                if td in rset:
                    # all-ScalarE tile: sum relu(S-m) and sum sign(S-m)
                    jr = spool.tile([128, DT_W], BF16, name="jnk")
                    nc.scalar.activation(
                        out=jr[:], in_=T[j][:], func=ACT.Relu,
                        bias=bias_nm[:], scale=1.0,
                        accum_out=accE[:, colR[td]:colR[td] + 1],
                    )
                    js = spool.tile([128, DT_W], BF16, name="jnk")
                    nc.scalar.activation(
                        out=js[:], in_=T[j][:], func=ACT.Sign,
                        bias=bias_nm[:], scale=1.0,
                        accum_out=accE[:, colS[td]:colS[td] + 1],
                    )
                    continue
                # copy S PSUM -> SBUF bf16 on ScalarE
                Sb = spool.tile([128, DT_W], BF16, name="scp")
                nc.scalar.activation(
                    out=Sb[:], in_=T[j][:], func=ACT.Copy,
                    bias=0.0, scale=1.0,
                )
                # b = (S > margin) * S ; accum = row-sum(b)
                bt = bpool.tile([128, DT_W], BF16, name="btile")
                nc.vector.scalar_tensor_tensor(
                    out=bt[:],
                    in0=Sb[:],
                    scalar=MARGIN,
                    in1=Sb[:],
                    op0=AL.is_gt,
                    op1=AL.mult,
                    accum_out=accD[:, stt_col[td]:stt_col[td] + 1],
                )
                for (wi, lo, w, tco) in wins_by_td.get(td, []):
                    m = wpool.tile([128, w], BF16, name="mask")
                    nc.vector.tensor_scalar(
                        out=m[:],
                        in0=tcb[:, tco:tco + w],
                        scalar1=trows[:, mt:mt + 1],
                        scalar2=None,
                        op0=AL.is_equal,
                    )
                    at = wpool.tile([128, w], BF16, name="atile")
                    nc.scalar.activation(
                        out=at[:],
                        in_=Sb[:, lo:lo + w],
                        func=ACT.Relu,
                        bias=1.0,
                        scale=-1.0,
                    )
                    ja = wpool.tile([128, w], BF16, name="junka")
                    nc.vector.scalar_tensor_tensor(
                        out=ja[:],
                        in0=at[:],
                        scalar=0.0,
                        in1=m[:],
                        op0=AL.add,
                        op1=AL.mult,
                        accum_out=accD[:, colA[wi]:colA[wi] + 1],
                    )
                    jb = wpool.tile([128, w], BF16, name="junkb")
                    nc.vector.scalar_tensor_tensor(
                        out=jb[:],
                        in0=bt[:, lo:lo + w],
                        scalar=0.0,
                        in1=m[:],
                        op0=AL.add,
                        op1=AL.mult,
                        accum_out=accD[:, colB[wi]:colB[wi] + 1],
                    )



def build_program(pad, relu_tiles=RELU_TILES, repeats=1):
    assert 0 < pad <= 96, f"label run too long for band kernel (pad={pad})"
    nc = bacc.Bacc()
    xt_d = nc.dram_tensor("xt", [2, 128, N_TOTAL], BF16, kind="ExternalInput")
    tcol_d = nc.dram_tensor("tcol", [N_TOTAL], F32, kind="ExternalInput")

    wins, a_len = _band_windows(pad)
    order = [(mt, dt) for mt in range(M_TILES) for dt in range(N_DT)]
    n_tiles = len(order)
    forced = {(mt, dt) for (mt, dt, _, _, _) in wins}
    nonforced = [td for td in order if td not in forced]
    rset = {
        nonforced[(i * len(nonforced)) // relu_tiles] for i in range(relu_tiles)
    } if relu_tiles else set()

    # accD columns: one per C-tile (b-sum), then 2 per band window.
    # accE columns: 2 per R-tile (relu-sum, sign-sum).
    cD = 0
    cE = 0
    stt_col = {}
    colR = {}
    colS = {}
    for td in order:
        if td in rset:
            colR[td] = cE
            colS[td] = cE + 1
            cE += 2
        else:
            stt_col[td] = cD
            cD += 1
    colA = {}
    colB = {}
    for wi in range(len(wins)):
        colA[wi] = cD
        colB[wi] = cD + 1
        cD += 2
    CD, CE = cD, cE

    out_d = nc.dram_tensor("out", [128, CD + CE], F32, kind="ExternalOutput")

    wins_by_td = {}
    for wi, (mt, dt, lo, w, tco) in enumerate(wins):
        wins_by_td.setdefault((mt, dt), []).append((wi, lo, w, tco))

    AL = mybir.AluOpType
    ACT = mybir.ActivationFunctionType

    with tile.TileContext(nc) as tc:
        with (
            tc.tile_pool(name="resident", bufs=1) as rpool,
            tc.tile_pool(name="psum", bufs=4, space="PSUM") as psum,
            tc.tile_pool(name="scopy", bufs=4) as spool,
            tc.tile_pool(name="bt", bufs=3) as bpool,
            tc.tile_pool(name="band", bufs=2) as wpool,
        ):
            # resident bf16 X^T (rolled), K split into 2 partition tiles
            xk = [rpool.tile([128, N_TOTAL], BF16, name=f"xk{k}") for k in range(2)]
            for ch in range(4):
                sl = slice(ch * 2048, (ch + 1) * 2048)
                for k in range(2):
                    nc.sync.dma_start(out=xk[k][:, sl], in_=xt_d[k, :, sl])

            # label tiles
            tcol_ap = tcol_d[:]
            tcb = rpool.tile([128, a_len + pad], F32, name="tcb")
            nc.sync.dma_start(
                out=tcb[:, 0:a_len],
                in_=bass.AP(tensor=tcol_ap.tensor, offset=0, ap=[[0, 128], [1, a_len]]),
            )
            nc.sync.dma_start(
                out=tcb[:, a_len:a_len + pad],
                in_=bass.AP(
                    tensor=tcol_ap.tensor,
                    offset=N_TOTAL - pad,
                    ap=[[0, 128], [1, pad]],
                ),
            )
            trows = rpool.tile([128, M_TILES], F32, name="trows")
            nc.sync.dma_start(
                out=trows[:],
                in_=bass.AP(
                    tensor=tcol_ap.tensor, offset=0, ap=[[1, 128], [128, M_TILES]]
                ),
            )

            accD = rpool.tile([128, CD], F32, name="accD")
            accE = rpool.tile([128, max(CE, 1)], F32, name="accE")
            nc.vector.memset(accD[:], 0.0)
            nc.vector.memset(accE[:], 0.0)
            bias_nm = rpool.tile([128, 1], F32, name="bias_nm")
            nc.vector.memset(bias_nm[:], -MARGIN)

            import contextlib
            loop_cm = tc.For_i(0, repeats, 1) if repeats > 1 else contextlib.nullcontext()
            with loop_cm:
                _main_body(nc, tc, psum, spool, bpool, wpool, xk, tcb, trows,
                           accD, accE, bias_nm, rset, stt_col, colR, colS,
                           colA, colB, wins_by_td, AL, ACT)

            nc.sync.dma_start(out=out_d[:, 0:CD], in_=accD[:])
            if CE:
                nc.sync.dma_start(out=out_d[:, CD:CD + CE], in_=accE[:])


    meta = dict(
        CD=CD, CE=CE, n_relu=len(rset),
        stt_cols=sorted(stt_col.values()),
        a_cols=sorted(colA.values()),
        b_cols=sorted(colB.values()),
        r_cols=sorted(colR.values()),
        s_cols=sorted(colS.values()),
    )
    return nc, meta


def host_reduce(out_arr, meta):
    """out_arr: [128, CD+CE] f32 from one core -> fp64 partial of loss*n."""
    a = out_arr.astype(np.float64)
    d = a[:, :meta["CD"]]
    tot = d[:, meta["stt_cols"]].sum()
    tot += d[:, meta["a_cols"]].sum()
    tot -= d[:, meta["b_cols"]].sum()
    if meta["CE"]:
        e = a[:, meta["CD"]:meta["CD"] + meta["CE"]]
        tot += e[:, meta["r_cols"]].sum()
        npix = meta["n_relu"] * 128 * DT_W
        tot += MARGIN * 0.5 * (npix + e[:, meta["s_cols"]].sum())
    return tot


def prepare_inputs(inputs, targets):
    X = np.asarray(inputs, dtype=np.float32)
    t = np.asarray(targets).astype(np.int64).reshape(-1)
    n, d = X.shape
    assert (n, d) == (N_TOTAL, D), f"kernel hardcoded for {N_TOTAL}x{D}, got {n}x{d}"
    perm = np.argsort(t, kind="stable")
    ts_ = t[perm]
    tf = ts_.astype(np.float32)
    bounds = np.flatnonzero(np.concatenate(([True], ts_[1:] != ts_[:-1], [True])))
    maxrun = int(np.diff(bounds).max())
    pad = int(-(-max(32, maxrun - 1) // 32) * 32)
    XT = np.ascontiguousarray(X[perm].T).astype(ml_dtypes.bfloat16)
    xt_full = XT.reshape(2, 128, N_TOTAL)
    in_maps = []
    for c in range(N_CORES):
        r = -c * ROWS
        in_maps.append({
            "xt": np.ascontiguousarray(np.roll(xt_full, r, axis=2)),
            "tcol": np.ascontiguousarray(np.roll(tf, r)),
        })
    return in_maps, pad


def run(inputs, targets, trace=False):
    in_maps, pad = prepare_inputs(inputs, targets)
    nc, meta = build_program(pad)
    nc.finalize()
    res = run_bass_kernel_spmd(
        nc, in_maps, core_ids=list(range(N_CORES)), trace=trace
    )
    total = 0.0
    for r in res.results:
        total += host_reduce(r["out"], meta)
    return np.asarray(total / N_TOTAL, dtype=np.float32), res


def kernel(inputs, targets):
    val, _ = run(inputs, targets, trace=False)
    return val



# revision 18
# speedup vs baseline: 4.1467x; 4.1467x over previous
"""Contrastive loss on 8 Trainium2 NeuronCores (Bass/Tile) — symmetric scheme.

loss * n = sum_ij f(s_ij),  s = <x_i, x_j>,
f = same ? relu(1-s) : (s > m) * s        (diagonal contributes 0: s_ii ~ 256)

Since f and the label mask are symmetric in (i, j), each unordered block-pair
of the 8x8 grid of 1024-row blocks is computed ONCE (weight 2; the diagonal
128x128 tiles weight 1), nearly halving matmul and pointwise work vs the full
matrix:

  - Host: sort rows by label (same-label pairs land in a band of width
    pad <= 128 around the diagonal), cast X^T to bf16, roll columns per core
    so core c's rows are always at columns 0..1023. Stage only columns
    [0, 5120): the diagonal block, blocks at distance 1,2,3, and HALF of the
    distance-4 block (the two column halves are swapped at staging time for
    cores 4-7, which makes the per-core program uniform while covering each
    distance-4 quadrant exactly once).
  - Device, per row-tile mt (128 rows): S slices via bf16 matmuls (PSUM f32):
      diag piece  cols [128mt, 1024)            weight 2 (first 128: weight 1)
      bulk tiles  cols [1024,2048),[2048,3072),[3072,4096) and a 512-wide
                  half-block (mt<4: [4096,4608), else [4608,5120))  weight 2
    Pointwise decomposition: sum b(s) with b = (s>m)s = relu(s-m) + m*(s>m):
      bulk "A" tiles: ScalarE relu(s-m) PSUM->SBUF bf16 with accumulated
        row-sum; DVE counts (r>0) with a 4x-mode tensor_scalar accum.
      bulk "V" tiles: DVE does the relu directly from PSUM (1x) to offload
        ScalarE; count as above.  (Static schedule balances the engines.)
      diag piece: ScalarE copies S to SBUF bf16 (band needs s values), DVE
        computes relu/count sums over the piece and over the weight-1 tile,
        and the same-label band correction sum mask*(relu(1-s) - b) over the
        first 128+pad columns via a short DVE chain (masks precomputed once
        from the labels).
  - Host: fp64 combine of per-(core,mt) accumulator columns, divide by n.
"""

import numpy as np
import ml_dtypes

import concourse.bass as bass
import concourse.mybir as mybir
from concourse import bacc
import concourse.tile as tile
from concourse.bass_utils import run_bass_kernel_spmd

N_TOTAL = 8192
D = 256
N_CORES = 8
ROWS = 1024
M_TILES = 8
STAGED = 5120
MARGIN = 0.3
F32 = mybir.dt.float32
BF16 = mybir.dt.bfloat16

# bulk tiles per mt: (column start | None for the mt-dependent half-block, width)
BULK = [(1024, 1024), (2048, 1024), (3072, 1024), (None, 512)]

# bulk tiles handled relu-on-DVE (path "V") instead of relu-on-ScalarE ("A").
P3_SET = {(0, 1), (1, 2), (2, 1), (3, 2), (4, 1), (5, 2), (6, 1), (7, 1)}
# bulk tiles whose count op runs on gpsimd (Pool) instead of DVE. The Pool
# engine in this toolchain rejects accumulating TensorScalar and all
# scalar_tensor_tensor ops, so this stays empty; Pool instead runs the band's
# plain elementwise ops (POOL_BAND).
POOL_CNT = set()
POOL_BAND = True
# engine for the diag-piece S copies ("pool" is ILLEGAL on real HW: GPSIMD
# instructions cannot access PSUM — the BIR verifier rejects them).
COPY_ENG = "act"

COLS_PER_MT = 12  # bsum_p, bsum_d, g_all, g_d, 4x(relu_or_bsum_t, cnt_t)


def build_program(pad, p3_set=None, pool_cnt=None, repeats=1, hw_loop=True,
                  copy_eng=COPY_ENG, pool_band=None):
    assert 0 < pad <= 128, f"label run too long for band kernel (pad={pad})"
    if p3_set is None:
        p3_set = P3_SET
    if pool_cnt is None:
        pool_cnt = POOL_CNT
    if pool_band is None:
        pool_band = POOL_BAND
    W_b = 128 + pad
    a_len = ROWS + pad
    CD = COLS_PER_MT * M_TILES

    nc = bacc.Bacc()
    xt_d = nc.dram_tensor("xt", [2, 128, STAGED], BF16, kind="ExternalInput")
    tcol_d = nc.dram_tensor("tcol", [a_len], F32, kind="ExternalInput")
    out_d = nc.dram_tensor("out", [128, CD], F32, kind="ExternalOutput")

    AL = mybir.AluOpType
    ACT = mybir.ActivationFunctionType

    def col(mt, j):
        return slice(mt * COLS_PER_MT + j, mt * COLS_PER_MT + j + 1)

    with tile.TileContext(nc) as tc:
        with (
            tc.tile_pool(name="resident", bufs=1) as rpool,
            tc.tile_pool(name="psum", bufs=4, space="PSUM") as psum,
            tc.tile_pool(name="sb", bufs=2) as spool,       # diag S copies
            tc.tile_pool(name="rb", bufs=2) as rbpool,      # diag relu vals
            tc.tile_pool(name="rt", bufs=3) as rtpool,      # bulk relu vals
            tc.tile_pool(name="jk", bufs=3) as jkpool,      # count junk outs
            tc.tile_pool(name="bd", bufs=2) as bdpool,      # band temps
        ):
            xk = [rpool.tile([128, STAGED], BF16, name=f"xk{k}") for k in range(2)]
            for k in range(2):
                for lo, hi in ((0, 2048), (2048, 4096), (4096, STAGED)):
                    nc.sync.dma_start(out=xk[k][:, lo:hi], in_=xt_d[k, :, lo:hi])

            tcol_ap = tcol_d[:]
            tcb = rpool.tile([128, a_len], F32, name="tcb")
            nc.sync.dma_start(
                out=tcb[:],
                in_=bass.AP(tensor=tcol_ap.tensor, offset=0, ap=[[0, 128], [1, a_len]]),
            )
            trows = rpool.tile([128, M_TILES], F32, name="trows")
            nc.sync.dma_start(
                out=trows[:],
                in_=bass.AP(
                    tensor=tcol_ap.tensor, offset=0, ap=[[1, 128], [128, M_TILES]]
                ),
            )

            accD = rpool.tile([128, CD], F32, name="accD")
            nc.vector.memset(accD[:], 0.0)
            b_negm = rpool.tile([128, 1], F32, name="b_negm")
            nc.vector.memset(b_negm[:], -MARGIN)

            # label-equality band masks, one per row-tile (input-only precompute)
            masks = []
            for mt in range(M_TILES):
                m = rpool.tile([128, W_b], BF16, name=f"mask{mt}")
                nc.vector.tensor_scalar(
                    out=m[:],
                    in0=tcb[:, mt * 128:mt * 128 + W_b],
                    scalar1=trows[:, mt:mt + 1],
                    scalar2=None,
                    op0=AL.is_equal,
                )
                masks.append(m)

            def body():
                for mt in range(M_TILES):
                    W_d = 1024 - mt * 128
                    lhs = [xk[k][:, mt * 128:mt * 128 + 128] for k in range(2)]
                    b4c0 = 4096 if mt < 4 else 4608

                    # --- matmuls, grouped so LDWEIGHTS amortizes over 2 tiles
                    # uniform [128,1024] PSUM tiles (one pool tag, 2 banks each)
                    tiles = {}
                    tiles["d"] = psum.tile([128, 1024], F32, name="S")
                    for ti, (c0, w) in enumerate(BULK):
                        tiles[ti] = psum.tile([128, 1024], F32, name="S")
                    groups = [["d", 0], [1, 2], [3]]
                    tcol0 = {"d": mt * 128, 0: 1024, 1: 2048, 2: 3072, 3: b4c0}
                    twid = {"d": W_d, 0: 1024, 1: 1024, 2: 1024, 3: 512}
                    for grp in groups:
                        for k in range(2):
                            for t in grp:
                                w = twid[t]
                                for lo in range(0, w, 512):
                                    hi = min(lo + 512, w)
                                    nc.tensor.matmul(
                                        tiles[t][:, lo:hi],
                                        lhs[k],
                                        xk[k][:, tcol0[t] + lo:tcol0[t] + hi],
                                        start=(k == 0),
                                        stop=(k == 1),
                                    )

                    # --- diag piece pointwise
                    Pd = tiles["d"]
                    sb_w = W_d if mt < 7 else 128 + pad
                    Sb = spool.tile([128, sb_w], BF16, name="Sb")
                    if copy_eng == "pool":
                        nc.gpsimd.tensor_scalar(
                            out=Sb[:, 0:W_d], in0=Pd[:, 0:W_d], scalar1=0.0,
                            scalar2=None, op0=AL.add,
                        )
                    else:
                        nc.scalar.activation(
                            out=Sb[:, 0:W_d], in_=Pd[:, 0:W_d], func=ACT.Copy,
                            bias=0.0, scale=1.0,
                        )
                    if mt == 7:
                        # band spill: first pad cols of block 1 (bulk tile 0)
                        if copy_eng == "pool":
                            nc.gpsimd.tensor_scalar(
                                out=Sb[:, 128:128 + pad], in0=tiles[0][:, 0:pad],
                                scalar1=0.0, scalar2=None, op0=AL.add,
                            )
                        else:
                            nc.scalar.activation(
                                out=Sb[:, 128:128 + pad], in_=tiles[0][:, 0:pad],
                                func=ACT.Copy, bias=0.0, scale=1.0,
                            )
                    # bv = (s>m)*s over the piece; accum -> piece b-sum.
                    # For mt=7 the piece is only [0:128); band values come from
                    # a second, non-accumulated stt over [0:W_b).
                    bv = rbpool.tile([128, max(W_d, W_b)], BF16, name="bv")
                    pw = W_d if mt < 7 else 128
                    nc.vector.scalar_tensor_tensor(
                        out=bv[:, 0:pw], in0=Sb[:, 0:pw], scalar=MARGIN,
                        in1=Sb[:, 0:pw], op0=AL.is_gt, op1=AL.mult,
                        accum_out=accD[:, col(mt, 0)],
                    )
                    if mt == 7:
                        nc.vector.scalar_tensor_tensor(
                            out=bv[:, 128:W_b], in0=Sb[:, 128:W_b], scalar=MARGIN,
                            in1=Sb[:, 128:W_b], op0=AL.is_gt, op1=AL.mult,
                        )
                    else:
                        # weight-1 diagonal 128-tile b-sum
                        jd = bdpool.tile([128, 128], BF16, name="jd")
                        nc.vector.scalar_tensor_tensor(
                            out=jd[:], in0=Sb[:, 0:128], scalar=MARGIN,
                            in1=Sb[:, 0:128], op0=AL.is_gt, op1=AL.mult,
                            accum_out=accD[:, col(mt, 1)],
                        )

                    # band correction: g = mask * (bt - at),  at = relu(1-s)
                    band_eng = nc.gpsimd if pool_band else nc.vector
                    an = bdpool.tile([128, W_b], BF16, name="an")
                    nc.vector.tensor_scalar(
                        out=an[:], in0=Sb[:, 0:W_b], scalar1=1.0, scalar2=0.0,
                        op0=AL.subtract, op1=AL.min,
                    )
                    e = bdpool.tile([128, W_b], BF16, name="e")
                    band_eng.tensor_tensor(
                        out=e[:], in0=an[:], in1=bv[:, 0:W_b], op=AL.add,
                    )
                    g = bdpool.tile([128, W_b], BF16, name="g")
                    band_eng.tensor_tensor(
                        out=g[:], in0=e[:], in1=masks[mt][:], op=AL.mult,
                    )
                    ja = bdpool.tile([128, W_b], BF16, name="ja")
                    nc.vector.tensor_scalar(
                        out=ja[:], in0=g[:], scalar1=0.0, scalar2=None,
                        op0=AL.add, op1=AL.add, accum_out=accD[:, col(mt, 2)],
                    )
                    jb = bdpool.tile([128, 128], BF16, name="jb")
                    nc.vector.tensor_scalar(
                        out=jb[:], in0=g[:, 0:128], scalar1=0.0, scalar2=None,
                        op0=AL.add, op1=AL.add, accum_out=accD[:, col(mt, 3)],
                    )

                    # --- bulk tiles pointwise
                    for ti, (c0, w) in enumerate(BULK):
                        P = tiles[ti][:, 0:w]
                        r = rtpool.tile([128, w], BF16, name=f"r{ti}")
                        if (mt, ti) in p3_set:
                            # DVE-only: relu from PSUM (1x), then summed at 4x.
                            # (dual-PSUM-input stt is rejected by the verifier)
                            nc.vector.tensor_scalar(
                                out=r[:], in0=P, scalar1=MARGIN, scalar2=0.0,
                                op0=AL.subtract, op1=AL.max,
                            )
                            js = jkpool.tile([128, w], BF16, name=f"s{ti}")
                            nc.vector.tensor_scalar(
                                out=js[:], in0=r[:], scalar1=0.0, scalar2=None,
                                op0=AL.add, op1=AL.add,
                                accum_out=accD[:, col(mt, 4 + 2 * ti)],
                            )
                        else:
                            # ScalarE relu(s-m) + accum; DVE counts (r>0)
                            nc.scalar.activation(
                                out=r[:], in_=P, func=ACT.Relu,
                                bias=b_negm[:], scale=1.0,
                                accum_out=accD[:, col(mt, 4 + 2 * ti)],
                            )
                        jt = jkpool.tile([128, w], BF16, name=f"j{ti}")
                        eng = nc.gpsimd if (mt, ti) in pool_cnt else nc.vector
                        eng.tensor_scalar(
                            out=jt[:], in0=r[:], scalar1=0.0, scalar2=None,
                            op0=AL.is_gt, op1=AL.add,
                            accum_out=accD[:, col(mt, 5 + 2 * ti)],
                        )

            import contextlib
            if repeats > 1 and not hw_loop:
                for _ in range(repeats):
                    body()
            else:
                loop_cm = (tc.For_i(0, repeats, 1) if repeats > 1
                           else contextlib.nullcontext())
                with loop_cm:
                    body()

            nc.sync.dma_start(out=out_d[:], in_=accD[:])

    meta = dict(pad=pad)
    return nc, meta


def host_reduce(out_arr, meta):
    """out_arr: [128, CD] f32 from one core -> fp64 share of loss*n."""
    a = out_arr.astype(np.float64)
    m = MARGIN
    total = 0.0
    for mt in range(M_TILES):
        c = a[:, mt * COLS_PER_MT:(mt + 1) * COLS_PER_MT].sum(axis=0)
        piece_b = c[0]
        dtile_b = c[1] if mt < 7 else piece_b
        # A-path tiles: col pair is (sum relu(s-m), count) -> + m*count;
        # V-path tiles: col pair is (sum b, 0) -> count col stays zero.
        bulk_b = sum(c[4 + 2 * ti] + m * c[5 + 2 * ti] for ti in range(4))
        g_all, g_d = c[2], c[3]
        total += 2.0 * (piece_b + bulk_b - g_all) - (dtile_b - g_d)
    return total


def prepare_inputs(inputs, targets):
    X = np.asarray(inputs, dtype=np.float32)
    t = np.asarray(targets).astype(np.int64).reshape(-1)
    n, d = X.shape
    assert (n, d) == (N_TOTAL, D), f"kernel hardcoded for {N_TOTAL}x{D}, got {n}x{d}"
    perm = np.argsort(t, kind="stable")
    ts_ = t[perm]
    tf = ts_.astype(np.float32)
    bounds = np.flatnonzero(np.concatenate(([True], ts_[1:] != ts_[:-1], [True])))
    maxrun = int(np.diff(bounds).max())
    pad = int(-(-max(32, maxrun - 1) // 32) * 32)
    a_len = ROWS + pad
    XT = np.ascontiguousarray(X[perm].T).astype(ml_dtypes.bfloat16)  # [256, 8192]
    in_maps = []
    for c in range(N_CORES):
        Xr = np.roll(XT, -c * ROWS, axis=1)[:, :STAGED]
        if c >= 4:
            Xr = Xr.copy()
            h1 = Xr[:, 4096:4608].copy()
            Xr[:, 4096:4608] = Xr[:, 4608:5120]
            Xr[:, 4608:5120] = h1
        tr = np.roll(tf, -c * ROWS)[:a_len]
        in_maps.append({
            "xt": np.ascontiguousarray(Xr.reshape(2, 128, STAGED)),
            "tcol": np.ascontiguousarray(tr),
        })
    return in_maps, pad


def run(inputs, targets, trace=False):
    in_maps, pad = prepare_inputs(inputs, targets)
    nc, meta = build_program(pad)
    nc.finalize()
    res = run_bass_kernel_spmd(
        nc, in_maps, core_ids=list(range(N_CORES)), trace=trace
    )
    total = 0.0
    for r in res.results:
        total += host_reduce(r["out"], meta)
    return np.asarray(total / N_TOTAL, dtype=np.float32), res


def kernel(inputs, targets):
    val, _ = run(inputs, targets, trace=False)
    return val
